# revision 36
# baseline (speedup 1.0000x reference)
"""Cross-modal attention Trainium2 kernel (fp8 DoubleRow, pair-bank evac).

Sharding: 8 cores, one per (direction, batch, query-half):
  core = dir*4 + b*2 + qh
  dir 0: out1 rows (q from x1, k/v from x2); dir 1: out2 (q from x2, k/v from x1)

All GEMMs run in fp8e4m3 DoubleRow mode (2 contraction k-tiles per
instruction):
  - q/k/v projections contract over c in pairs of 128-chunks (host ships x
    and W with c-chunk pairs interleaved on the free axis).
  - scores per head contract over D=64 using a broadcast second plane.
  - attn@v streams per key-pair (jp) within its own head with exp'd scores
    stationary; v is split hi+lo fp8 (residual) for ~bf16 value accuracy.
    A separate tiny matmul against a 1/64 column accumulates the softmax
    denominator.
  - out-proj: att (bf16) stationary after PE transposes, wo bf16 moving;
    final evac folds 1/64 and the output bias.

The per-head pacing item is PSUM evacuation on ACT+DVE. All evacuations are
batched to [128, 1024] f32 reads spanning two PSUM banks (score pairs over
the two query halves; projection pairs over adjacent output chunks), which
amortizes the fixed SBUF/PSUM access latency per instruction.

exp is split A/D: ACT runs native Exp (scale=1/8) straight to fp8e4; DVE
runs a quantized-Schraudolph bit trick:
  u8 = round(score * log2(e)/8 * 8 + 55.54)  viewed as fp8e4 bits
Weights are prescaled x16 on host (e4m3 subnormal avoidance); 1/16 folds
into the PSUM evacuations. The 1/sqrt(D) score scale folds into the exp.
"""

import sys

sys.path.insert(0, "/opt/trn_rl_repo")

import numpy as np
import ml_dtypes

EMBED = 512
H = 8
D = 64
B = 2
L = 2048
LQ = 1024

SW = 16.0  # weight prescale (host)
SA = 64.0  # att prescale via 1/SA ones-column
C1 = float(np.log2(np.e) * 8.0 / 8.0)  # schraudolph mult (incl 1/sqrt(D))
C2 = 56.0 - 0.46  # schraudolph magic bias

f8np = ml_dtypes.float8_e4m3
bfnp = ml_dtypes.bfloat16

_CACHE = {}

# exp engine per (h, j): A=ACT native, D=DVE schraudolph. 9 A : 7 D.
EXP_PAT = "AADADADADADADADA"
# kt / tp4 / y evac engine rotation
KT_PAT = "DA"
TEV_PAT = "DA"
YEV_PAT = "DA"


def _build_nc(debug=False):
    import concourse.bacc as bacc
    import concourse.mybir as mybir
    import concourse.tile as tile

    BF = mybir.dt.bfloat16
    F32 = mybir.dt.float32
    F8 = mybir.dt.float8e4
    U8 = mybir.dt.uint8
    EXP = mybir.ActivationFunctionType.Exp
    IDENT = mybir.ActivationFunctionType.Identity
    DR = mybir.MatmulPerfMode.DoubleRow
    AL = mybir.AluOpType

    nc = bacc.Bacc("TRN2", target_bir_lowering=False)

    xq_d = nc.dram_tensor("xq", [128, 2, 2, LQ], F8, kind="ExternalInput")
    xk_d = nc.dram_tensor("xk", [2, 2, 128, 2, L], F8, kind="ExternalInput")
    w1_d = nc.dram_tensor("w1", [128, 2, 2, 2, 512], F8, kind="ExternalInput")
    w2_d = nc.dram_tensor("w2", [128, 2, 2, 2, 512], F8, kind="ExternalInput")
    w3_d = nc.dram_tensor("w3", [128, 2, 2, 2, 512], BF, kind="ExternalInput")
    bqk_d = nc.dram_tensor("bqk", [128, 8], F32, kind="ExternalInput")
    bo2_d = nc.dram_tensor("bo2", [2, 512], BF, kind="ExternalInput")
    idn_d = nc.dram_tensor("idn", [128, 128], BF, kind="ExternalInput")
    y_d = nc.dram_tensor("y", [8, 128, 512], F32, kind="ExternalOutput")
    if debug:
        dqt_d = nc.dram_tensor("dqt", [128, 4, 2, LQ], F8, kind="ExternalOutput")
        dkt_d = nc.dram_tensor("dkt", [128, 4, L], F8, kind="ExternalOutput")
        dvh_d = nc.dram_tensor("dvh", [128, 8, 2, H, D + 1], F8, kind="ExternalOutput")
        dvl_d = nc.dram_tensor("dvl", [128, 8, 2, H, D + 1], F8, kind="ExternalOutput")
        datt_d = nc.dram_tensor("datt", [8, 128, H, D], BF, kind="ExternalOutput")
        datT_d = nc.dram_tensor("datT", [2, 128, 2, LQ], BF, kind="ExternalOutput")
        dex_d = nc.dram_tensor("dex", [8, 128, 2, LQ], F8, kind="ExternalOutput")
        davsb_d = nc.dram_tensor("davsb", [8, 128, H, D], BF, kind="ExternalOutput")
        drc_d = nc.dram_tensor("drc", [8, 128, 8], F32, kind="ExternalOutput")

    with tile.TileContext(nc) as tc:
        with tc.tile_pool(name="persist", bufs=1) as pp:
            xq = pp.tile([128, 2, 2, LQ], F8, name="xq")
            xk = pp.tile([128, 2, 2, 2, L], F8, name="xk")
            w1 = pp.tile([128, 2, 2, 2, 512], F8, name="w1")
            w2 = pp.tile([128, 2, 2, 2, 512], F8, name="w2")
            w3 = pp.tile([128, 2, 2, 2, 512], BF, name="w3")
            bqk = pp.tile([128, 8], F32, name="bqk")
            bo2 = pp.tile([2, 512], BF, name="bo2")
            ones_r = pp.tile([2, 128], BF, name="ones_r")
            idn = pp.tile([128, 128], BF, name="idn")
            qt = pp.tile([128, 4, 2, LQ], F8, name="qt")
            kt = pp.tile([128, 4, L], F8, name="kt")
            # v carries a 65th column (1/SA on hi, 0 on lo) so the attn@v
            # matmul accumulates the softmax denominator in-place.
            vh = pp.tile([128, 8, 2, H, D + 1], F8, name="vh")
            vl = pp.tile([128, 8, 2, H, D + 1], F8, name="vl")
            att = [pp.tile([128, H, D], BF, name=f"att{qc}") for qc in range(8)]
            atT = [pp.tile([128, 2, LQ], BF, name=f"atT{c}") for c in range(2)]
            wup = pp.tile([128, 512], BF, name="wup")

            # DMA order tuned so qk_proj(0)'s inputs land first, then the
            # first v seq-half, then the second halves of everything. xk
            # quadrants are split in halves so the first k-proj and v-proj
            # can start before the full 16KB of x-other lands.
            # All input DMAs ride the sync (SP) queue: DMA dispatch holds the
            # issuing sequencer through descriptor generation, so issuing
            # from scalar/vector would block ACT/DVE compute dispatch.
            ha, hb = slice(0, 1024), slice(1024, 2048)
            nc.sync.dma_start(out=bqk, in_=bqk_d[:])
            nc.sync.dma_start(out=w1, in_=w1_d[:])
            nc.sync.dma_start(out=xq, in_=xq_d[:])
            nc.sync.dma_start(out=xk[:, 0, 0, :, ha], in_=xk_d[0, 0, :, :, ha])
            nc.sync.dma_start(out=xk[:, 0, 1, :, ha], in_=xk_d[0, 1, :, :, ha])
            nc.sync.dma_start(out=xk[:, 1, 0, :, ha], in_=xk_d[1, 0, :, :, ha])
            nc.sync.dma_start(out=xk[:, 1, 1, :, ha], in_=xk_d[1, 1, :, :, ha])
            nc.sync.dma_start(out=w2, in_=w2_d[:])
            nc.sync.dma_start(out=xk[:, 0, 0, :, hb], in_=xk_d[0, 0, :, :, hb])
            nc.sync.dma_start(out=xk[:, 0, 1, :, hb], in_=xk_d[0, 1, :, :, hb])
            nc.sync.dma_start(out=xk[:, 1, 0, :, hb], in_=xk_d[1, 0, :, :, hb])
            nc.sync.dma_start(out=xk[:, 1, 1, :, hb], in_=xk_d[1, 1, :, :, hb])
            nc.gpsimd.memset(qt[:, :, 1, :], 0.0)
            nc.sync.dma_start(out=w3, in_=w3_d[:])
            nc.sync.dma_start(out=idn, in_=idn_d[:])
            nc.sync.dma_start(out=bo2, in_=bo2_d[:])
            nc.gpsimd.memset(ones_r, 1.0)
            nc.vector.memset(vh[:, :, :, :, D : D + 1], 1.0 / SA)
            nc.gpsimd.memset(vl[:, :, :, :, D : D + 1], 0.0)

            with (
                tc.tile_pool(name="scp2", bufs=3, space="PSUM") as scp2,
                tc.tile_pool(name="avp", bufs=2, space="PSUM") as avp,
                tc.tile_pool(name="exp", bufs=16) as expool,
                tc.tile_pool(name="nrm", bufs=4) as nrm,
                tc.tile_pool(name="yst", bufs=4) as yst,
            ):
                # prologue warmups: ACT Exp table preload + PE p-state ramp
                dm = nrm.tile([1, 2], F32, name="dm")
                nc.vector.memset(dm, 0.0)
                dm2 = nrm.tile([1, 2], F32, name="dm2")
                nc.scalar.activation(dm2, dm, EXP)
                nc.vector.memset(wup, 0.0)
                wps = scp2.tile([128, 512], F32, name="s2")
                for i in range(8):
                    nc.tensor.matmul(
                        wps, wup[:, 0:128], wup, start=(i == 0), stop=(i == 7)
                    )

                ex_t = {}
                av_t = {}

                def qk_proj(f):
                    # q-dims chunk f: one paired evac over both query halves
                    ps = scp2.tile([128, 1024], F32, name="s2")
                    for ih in range(2):
                        for cp in range(2):
                            nc.tensor.matmul(
                                ps[:, ih * 512 : (ih + 1) * 512],
                                w1[:, 0, cp, :, f * 128 : (f + 1) * 128],
                                xq[:, cp, :, ih * 512 : (ih + 1) * 512],
                                start=(cp == 0),
                                stop=(cp == 1),
                                perf_mode=DR,
                            )
                    nc.scalar.activation(
                        qt[:, f, 0, :],
                        ps,
                        IDENT,
                        bias=bqk[:, f : f + 1],
                        scale=1.0 / SW,
                    )
                def kt_proj(f, tp):
                    # k-dims chunk f, th pair tp: one paired evac
                    ps = scp2.tile([128, 1024], F32, name="s2")
                    for t2 in range(2):
                        th = tp * 2 + t2
                        for cp in range(2):
                            nc.tensor.matmul(
                                ps[:, t2 * 512 : (t2 + 1) * 512],
                                w1[:, 1, cp, :, f * 128 : (f + 1) * 128],
                                xk[:, 0, cp, :, th * 512 : (th + 1) * 512],
                                start=(cp == 0),
                                stop=(cp == 1),
                                perf_mode=DR,
                            )
                    # k bias is softmax-invariant (constant per query): skip
                    dst = kt[:, f, tp * 1024 : (tp + 1) * 1024]
                    if KT_PAT[(f * 2 + tp) % 2] == "D":
                        nc.vector.tensor_scalar(
                            dst, ps, 1.0 / SW, 0.0, AL.mult, AL.add
                        )
                    else:
                        nc.scalar.mul(dst, ps, 1.0 / SW)

                def v_proj(p):
                    # seq-chunk pair (2p, 2p+1): paired hi/lo evacs
                    ps = scp2.tile([128, 2, 512], F32, name="s2")
                    for pl in range(2):
                        t = p * 2 + pl
                        first = True
                        for xi, wi in ((0, 0), (0, 1), (1, 0)):
                            for cp in range(2):
                                nc.tensor.matmul(
                                    ps[:, pl, :],
                                    xk[:, xi, cp, :, t * 128 : (t + 1) * 128],
                                    w2[:, wi, cp],
                                    start=first,
                                    stop=(xi == 1 and cp == 1),
                                    perf_mode=DR,
                                )
                                first = False
                    psv = ps.rearrange("p a (h d) -> p a h d", h=H)
                    hi = vh[:, p, :, :, 0:D]
                    nc.scalar.mul(hi, psv, 1.0 / SW)
                    nc.vector.scalar_tensor_tensor(
                        vl[:, p, :, :, 0:D], psv, 1.0 / SW, hi, AL.mult, AL.subtract
                    )

                def score(h, j):
                    f, base = h // 2, 64 * (h % 2)
                    jp, pl = j // 2, j % 2
                    if pl == 0:
                        ex_t[(h, jp)] = expool.tile([128, 2, LQ], F8, name="ex")
                    ex = ex_t[(h, jp)]
                    lhs = kt[base : base + 64, f, j * 128 : (j + 1) * 128]
                    lhs = lhs.unsqueeze(1).broadcast_to((64, 2, 128))
                    ps = scp2.tile([128, 1024], F32, name="s2")
                    for ih in range(2):
                        nc.tensor.matmul(
                            ps[:, ih * 512 : (ih + 1) * 512],
                            lhs,
                            qt[base : base + 64, f, :, ih * 512 : (ih + 1) * 512],
                            start=True,
                            stop=True,
                            perf_mode=DR,
                        )
                    if EXP_PAT[j] == "A":
                        nc.scalar.activation(ex[:, pl, :], ps, EXP, scale=1.0 / 8.0)
                    else:
                        nc.vector.tensor_scalar(
                            ex.bitcast(U8)[:, pl, :], ps, C1, C2, AL.mult, AL.add
                        )

                def attnv_qc(h, qc):
                    # one PSUM accumulation region at a time per bank: the
                    # 16 matmuls of a qc region run as one sequential group
                    # (hardware allows only one open accumulation group per
                    # PSUM bank; interleaving regions corrupts the sums).
                    if qc == 0:
                        av_t[h] = [
                            avp.tile([128, 4, D + 1], F32, name="av")
                            for _ in range(2)
                        ]
                    out = av_t[h][qc // 4][:, qc % 4, :]
                    for jp in range(8):
                        ex = ex_t[(h, jp)]
                        if debug and h == 0 and qc == 0:
                            nc.sync.dma_start(out=dex_d[jp], in_=ex)
                        exq = ex[:, :, qc * 128 : (qc + 1) * 128]
                        for gi, vv in enumerate((vh, vl)):
                            nc.tensor.matmul(
                                out,
                                exq,
                                vv[:, jp, :, h, :],
                                start=(jp == 0 and gi == 0),
                                stop=(jp == 7 and gi == 1),
                                perf_mode=DR,
                            )
                    if qc == 7:
                        for jp in range(8):
                            del ex_t[(h, jp)]

                def norm(h):
                    rc = nrm.tile([128, 8], F32, name="rc")
                    avsb = nrm.tile([128, 8, D], BF, name="avsb")
                    for half in range(2):
                        av = av_t[h][half]
                        sl = slice(half * 4, half * 4 + 4)
                        nc.vector.reciprocal(rc[:, sl], av[:, :, D : D + 1])
                        nc.vector.tensor_copy(avsb[:, sl, :], av[:, :, 0:D])
                    del av_t[h]
                    if debug:
                        nc.sync.dma_start(out=davsb_d[h], in_=avsb)
                        nc.sync.dma_start(out=drc_d[h], in_=rc)
                    for qc in range(8):
                        nc.gpsimd.tensor_scalar(
                            att[qc][:, h, :],
                            avsb[:, qc, :],
                            rc[:, qc : qc + 1],
                            1.0,
                            AL.mult,
                            AL.mult,
                        )

                def tp4(cc, g):
                    # 4 transposes of head-pair cc, query chunks 4g..4g+3,
                    # evacuated in one [128, 512] op
                    tp = scp2.tile([128, 4, 128], BF, name="s2")
                    for q2 in range(4):
                        qc = g * 4 + q2
                        nc.tensor.transpose(
                            tp[:, q2, :], att[qc][:, 2 * cc : 2 * cc + 2, :], idn
                        )
                    dst = atT[cc // 2][:, cc % 2, g * 512 : (g + 1) * 512]
                    if TEV_PAT[(cc * 2 + g) % 2] == "D":
                        nc.vector.tensor_copy(dst, tp)
                    else:
                        nc.scalar.copy(dst, tp)

                def out_proj(i):
                    # query chunk pair (2i, 2i+1): paired evac + 2 DMAs
                    ps = scp2.tile([128, 2, 512], F32, name="s2")
                    for pl in range(2):
                        t = i * 2 + pl
                        k = 0
                        for cp in range(2):
                            for p2 in range(2):
                                nc.tensor.matmul(
                                    ps[:, pl, :],
                                    atT[cp][:, p2, t * 128 : (t + 1) * 128],
                                    w3[:, 0, cp, p2, :],
                                    start=(k == 0),
                                    stop=False,
                                )
                                k += 1
                        nc.tensor.matmul(
                            ps[:, pl, :], ones_r, bo2, start=False, stop=True
                        )
                    ysb = yst.tile([128, 2, 512], F32, name="ysb")
                    if YEV_PAT[i % 2] == "D":
                        nc.vector.tensor_scalar(
                            ysb, ps, 1.0 / SA, 0.0, AL.mult, AL.add
                        )
                    else:
                        nc.scalar.mul(ysb, ps, 1.0 / SA)
                    for pl in range(2):
                        nc.sync.dma_start(out=y_d[i * 2 + pl], in_=ysb[:, pl, :])

                # ---- schedule --------------------------------------------
                qk_proj(0)
                kt_proj(0, 0)
                vq = list(range(8))

                def drain_v(n):
                    for _ in range(min(n, len(vq))):
                        v_proj(vq.pop(0))

                # attnv for head h runs during head h+1 (its exps done: no
                # PE stalls), one qc burst per j-slot.
                tpq = []  # pending tp4 units

                def norm_head(hh):
                    norm(hh)
                    if hh % 2 == 1:
                        tpq.append((hh // 2, 0))
                        tpq.append((hh // 2, 1))

                # (head, j) -> deferred projection work, spread over heads
                PROJ = {
                    (0, 5): lambda: kt_proj(0, 1),
                    (0, 13): lambda: qk_proj(1),
                    (1, 3): lambda: kt_proj(1, 0),
                    (1, 9): lambda: kt_proj(1, 1),
                    (2, 5): lambda: qk_proj(2),
                    (3, 3): lambda: kt_proj(2, 0),
                    (3, 9): lambda: kt_proj(2, 1),
                    (4, 5): lambda: qk_proj(3),
                    (5, 3): lambda: kt_proj(3, 0),
                    (5, 9): lambda: kt_proj(3, 1),
                }

                AQC = {2: 0, 3: 1, 4: 2, 5: 3, 8: 4, 9: 5, 10: 6, 11: 7}
                for h in range(8):
                    for j in range(16):
                        score(h, j)
                        if (h, j) in PROJ:
                            PROJ[(h, j)]()
                        if h == 0 and j in (3, 7, 9, 11):
                            drain_v(2)
                        if h >= 1 and j in AQC:
                            attnv_qc(h - 1, AQC[j])
                        if h >= 1 and j == 13:
                            norm_head(h - 1)
                        if j in (6, 12) and tpq:
                            tp4(*tpq.pop(0))
                # tail: last head's attnv + norm, transposes, out-proj
                for qc in range(8):
                    attnv_qc(7, qc)
                norm_head(7)
                assert not vq
                for cc, g in tpq:
                    tp4(cc, g)
                for i in range(4):
                    out_proj(i)
                if debug:
                    nc.sync.dma_start(out=dqt_d[:], in_=qt)
                    nc.sync.dma_start(out=dkt_d[:], in_=kt)
                    nc.sync.dma_start(out=dvh_d[:], in_=vh)
                    nc.sync.dma_start(out=dvl_d[:], in_=vl)
                    for qc in range(8):
                        nc.sync.dma_start(out=datt_d[qc], in_=att[qc])
                    for c in range(2):
                        nc.sync.dma_start(out=datT_d[c], in_=atT[c])

    nc.finalize()
    return nc


def _bo2(b):
    hi = b.astype(bfnp)
    lo = (b - hi.astype(np.float32)).astype(bfnp)
    return np.stack([hi, lo], axis=0).reshape(2, 512)


def _pairplane(a):
    # [512, N] -> [2, 128, 2, N] with c-chunk pairs interleaved on planes
    n = a.shape[1]
    return np.ascontiguousarray(a.reshape(2, 2, 128, n).transpose(0, 2, 1, 3))


def _prep_weights(qkv_w, qkv_b, out_w, out_b):
    w = qkv_w.reshape(H, 3, D, EMBED)
    b3 = qkv_b.reshape(H, 3, D)
    wq = w[:, 0].reshape(EMBED, EMBED)
    wk = w[:, 1].reshape(EMBED, EMBED)
    wv = w[:, 2].reshape(EMBED, EMBED)
    bq = b3[:, 0].reshape(EMBED)
    bv = b3[:, 2].reshape(EMBED)

    def hilo(m):
        hi = (m * SW).astype(f8np)
        lo = (m * SW - hi.astype(np.float32)).astype(f8np)
        return hi, lo

    wq8 = (wq.T.astype(np.float32) * SW).astype(f8np)
    wk8 = (wk.T.astype(np.float32) * SW).astype(f8np)
    wvh, wvl = hilo(wv.T.astype(np.float32))
    wo16 = out_w.T.astype(np.float32).astype(bfnp)

    def pack2(a, b):
        # two [512, 512] f8 -> [128, 2(which), 2(cp), 2(plane), 512]
        s = np.stack([_pairplane(a), _pairplane(b)], axis=0)  # [w, cp, 128, pl, f]
        return np.ascontiguousarray(s.transpose(2, 0, 1, 3, 4))

    bqk = np.zeros((128, 8), np.float32)
    for f in range(4):
        bqk[:, f] = bq[f * 128 : (f + 1) * 128]
    return {
        "w1": pack2(wq8, wk8),
        "w2": pack2(wvh, wvl),
        "w3": pack2(wo16, wo16),
        "bqk": bqk,
        "bo2": _bo2((out_b + out_w @ bv).astype(np.float32) * SA),
        "idn": np.eye(128, dtype=np.float32).astype(bfnp),
    }


def _make_in_maps(x1, x2, shared):
    xT = {}
    for mod, x in ((0, x1), (1, x2)):
        for b in range(B):
            t = np.ascontiguousarray(x[b].T).astype(np.float32)
            hi = t.astype(f8np)
            lo = (t - hi.astype(np.float32)).astype(f8np)
            xT[(mod, b)] = (hi, lo)
    in_maps = []
    for core in range(8):
        d, b, qh = core // 4, (core // 2) % 2, core % 2
        hi_q = xT[(d, b)][0][:, qh * LQ : (qh + 1) * LQ]
        hi_kv, lo_kv = xT[(1 - d, b)]
        m = dict(shared)
        m["xq"] = np.ascontiguousarray(
            _pairplane(np.ascontiguousarray(hi_q)).transpose(1, 0, 2, 3)
        )
        m["xk"] = np.ascontiguousarray(
            np.stack([_pairplane(hi_kv), _pairplane(lo_kv)], axis=0)
        )
        in_maps.append(m)
    return in_maps


def kernel(x1, x2, qkv_w, qkv_b, out_w, out_b):
    from concourse.bass_utils import run_bass_kernel_spmd

    x1 = np.asarray(x1, dtype=np.float32)
    x2 = np.asarray(x2, dtype=np.float32)
    shared = _prep_weights(
        np.asarray(qkv_w, np.float32),
        np.asarray(qkv_b, np.float32),
        np.asarray(out_w, np.float32),
        np.asarray(out_b, np.float32),
    )
    in_maps = _make_in_maps(x1, x2, shared)

    if "nc" not in _CACHE:
        _CACHE["nc"] = _build_nc()
    try:
        res = run_bass_kernel_spmd(_CACHE["nc"], in_maps, core_ids=list(range(8)))
    except Exception:
        res = run_bass_kernel_spmd(_CACHE["nc"], in_maps, core_ids=list(range(8)))

    out1 = np.empty((B, L, EMBED), np.float32)
    out2 = np.empty((B, L, EMBED), np.float32)
    outs = {0: out1, 1: out2}
    for core in range(8):
        d, b, qh = core // 4, (core // 2) % 2, core % 2
        yc = res.results[core]["y"].reshape(LQ, EMBED)
        outs[d][b, qh * LQ : (qh + 1) * LQ, :] = yc
    return out1, out2


# revision 40
# speedup vs baseline: 1.0559x; 1.0559x over previous
"""Cross-modal attention Trainium2 kernel (fp8 DoubleRow, pair-bank evac).

Sharding: 8 cores, one per (direction, batch, query-half):
  core = dir*4 + b*2 + qh
  dir 0: out1 rows (q from x1, k/v from x2); dir 1: out2 (q from x2, k/v from x1)

All GEMMs run in fp8e4m3 DoubleRow mode (2 contraction k-tiles per
instruction):
  - q/k/v projections contract over c in pairs of 128-chunks (host ships x
    and W with c-chunk pairs interleaved on the free axis).
  - scores per head contract over D=64 using a broadcast second plane.
  - attn@v streams per key-pair (jp) within its own head with exp'd scores
    stationary; v is split hi+lo fp8 (residual) for ~bf16 value accuracy.
    A separate tiny matmul against a 1/64 column accumulates the softmax
    denominator.
  - out-proj: att (bf16) stationary after PE transposes, wo bf16 moving;
    final evac folds 1/64 and the output bias.

The per-head pacing item is PSUM evacuation on ACT+DVE. All evacuations are
batched to [128, 1024] f32 reads spanning two PSUM banks (score pairs over
the two query halves; projection pairs over adjacent output chunks), which
amortizes the fixed SBUF/PSUM access latency per instruction.

exp is split A/D: ACT runs native Exp (scale=1/8) straight to fp8e4; DVE
runs a quantized-Schraudolph bit trick:
  u8 = round(score * log2(e)/8 * 8 + 55.54)  viewed as fp8e4 bits
Weights are prescaled x16 on host (e4m3 subnormal avoidance); 1/16 folds
into the PSUM evacuations. The 1/sqrt(D) score scale folds into the exp.
"""

import sys

sys.path.insert(0, "/opt/trn_rl_repo")

import numpy as np
import ml_dtypes

EMBED = 512
H = 8
D = 64
B = 2
L = 2048
LQ = 1024

SW = 16.0  # weight prescale (host)
SA = 64.0  # att prescale via 1/SA ones-column
C1 = float(np.log2(np.e) * 8.0 / 8.0)  # schraudolph mult (incl 1/sqrt(D))
C2 = 56.0 - 0.46  # schraudolph magic bias

f8np = ml_dtypes.float8_e4m3
bfnp = ml_dtypes.bfloat16

_CACHE = {}

# exp engine per (h, j): A=ACT native, D=DVE schraudolph. 9 A : 7 D.
EXP_PAT = "AADADADADADADADA"
# kt / tp4 / y evac engine rotation
KT_PAT = "DA"
TEV_PAT = "DA"
YEV_PAT = "DA"


def _build_nc(debug=False):
    import concourse.bacc as bacc
    import concourse.mybir as mybir
    import concourse.tile as tile

    BF = mybir.dt.bfloat16
    F32 = mybir.dt.float32
    F8 = mybir.dt.float8e4
    U8 = mybir.dt.uint8
    EXP = mybir.ActivationFunctionType.Exp
    IDENT = mybir.ActivationFunctionType.Identity
    DR = mybir.MatmulPerfMode.DoubleRow
    AL = mybir.AluOpType

    nc = bacc.Bacc("TRN2", target_bir_lowering=False)

    xq_d = nc.dram_tensor("xq", [128, 2, 2, LQ], F8, kind="ExternalInput")
    xk_d = nc.dram_tensor("xk", [2, 2, 128, 2, L], F8, kind="ExternalInput")
    w1_d = nc.dram_tensor("w1", [128, 2, 2, 2, 512], F8, kind="ExternalInput")
    w2_d = nc.dram_tensor("w2", [128, 2, 2, 2, 512], F8, kind="ExternalInput")
    w3_d = nc.dram_tensor("w3", [128, 2, 2, 2, 512], BF, kind="ExternalInput")
    bqk_d = nc.dram_tensor("bqk", [128, 8], F32, kind="ExternalInput")
    bo2_d = nc.dram_tensor("bo2", [2, 512], BF, kind="ExternalInput")
    idn_d = nc.dram_tensor("idn", [128, 128], BF, kind="ExternalInput")
    y_d = nc.dram_tensor("y", [8, 128, 512], F32, kind="ExternalOutput")
    if debug:
        dqt_d = nc.dram_tensor("dqt", [128, 4, 2, LQ], F8, kind="ExternalOutput")
        dkt_d = nc.dram_tensor("dkt", [128, 4, L], F8, kind="ExternalOutput")
        dvh_d = nc.dram_tensor("dvh", [128, 8, 2, H, D + 1], F8, kind="ExternalOutput")
        dvl_d = nc.dram_tensor("dvl", [128, 8, 2, H, D + 1], F8, kind="ExternalOutput")
        datt_d = nc.dram_tensor("datt", [8, 128, H, D], BF, kind="ExternalOutput")
        datT_d = nc.dram_tensor("datT", [2, 128, 2, LQ], BF, kind="ExternalOutput")
        dex_d = nc.dram_tensor("dex", [8, 128, 2, LQ], F8, kind="ExternalOutput")
        davsb_d = nc.dram_tensor("davsb", [8, 128, H, D], BF, kind="ExternalOutput")
        drc_d = nc.dram_tensor("drc", [8, 128, 8], F32, kind="ExternalOutput")

    with tile.TileContext(nc) as tc:
        with tc.tile_pool(name="persist", bufs=1) as pp:
            xq = pp.tile([128, 2, 2, LQ], F8, name="xq")
            xk = pp.tile([128, 2, 2, 2, L], F8, name="xk")
            w1 = pp.tile([128, 2, 2, 2, 512], F8, name="w1")
            w2 = pp.tile([128, 2, 2, 2, 512], F8, name="w2")
            w3 = pp.tile([128, 2, 2, 2, 512], BF, name="w3")
            bqk = pp.tile([128, 8], F32, name="bqk")
            bo2 = pp.tile([2, 512], BF, name="bo2")
            ones_r = pp.tile([2, 128], BF, name="ones_r")
            idn = pp.tile([128, 128], BF, name="idn")
            qt = pp.tile([128, 4, 2, LQ], F8, name="qt")
            kt = pp.tile([128, 4, L], F8, name="kt")
            # v carries a 65th column (1/SA on hi, 0 on lo) so the attn@v
            # matmul accumulates the softmax denominator in-place.
            vh = pp.tile([128, 8, 2, H, D + 1], F8, name="vh")
            vl = pp.tile([128, 8, 2, H, D + 1], F8, name="vl")
            att = [pp.tile([128, H, D], BF, name=f"att{qc}") for qc in range(8)]
            atT = [pp.tile([128, 2, LQ], BF, name=f"atT{c}") for c in range(2)]
            wup = pp.tile([128, 512], BF, name="wup")

            # DMA order tuned so qk_proj(0)'s inputs land first, then the
            # first v seq-half, then the second halves of everything. xk
            # quadrants are split in halves so the first k-proj and v-proj
            # can start before the full 16KB of x-other lands.
            # All input DMAs ride the sync (SP) queue: DMA dispatch holds the
            # issuing sequencer through descriptor generation, so issuing
            # from scalar/vector would block ACT/DVE compute dispatch.
            ha, hb = slice(0, 1024), slice(1024, 2048)
            nc.sync.dma_start(out=bqk, in_=bqk_d[:])
            nc.sync.dma_start(out=w1, in_=w1_d[:])
            nc.sync.dma_start(out=xq, in_=xq_d[:])
            nc.sync.dma_start(out=xk[:, 0, 0, :, ha], in_=xk_d[0, 0, :, :, ha])
            nc.sync.dma_start(out=xk[:, 0, 1, :, ha], in_=xk_d[0, 1, :, :, ha])
            nc.sync.dma_start(out=xk[:, 0, 0, :, hb], in_=xk_d[0, 0, :, :, hb])
            nc.sync.dma_start(out=xk[:, 0, 1, :, hb], in_=xk_d[0, 1, :, :, hb])
            nc.sync.dma_start(out=w2, in_=w2_d[:])
            nc.sync.dma_start(out=xk[:, 1, 0, :, ha], in_=xk_d[1, 0, :, :, ha])
            nc.sync.dma_start(out=xk[:, 1, 1, :, ha], in_=xk_d[1, 1, :, :, ha])
            nc.sync.dma_start(out=xk[:, 1, 0, :, hb], in_=xk_d[1, 0, :, :, hb])
            nc.sync.dma_start(out=xk[:, 1, 1, :, hb], in_=xk_d[1, 1, :, :, hb])
            nc.gpsimd.memset(qt[:, :, 1, :], 0.0)
            nc.sync.dma_start(out=w3, in_=w3_d[:])
            nc.sync.dma_start(out=idn, in_=idn_d[:])
            nc.sync.dma_start(out=bo2, in_=bo2_d[:])
            nc.gpsimd.memset(ones_r, 1.0)
            nc.vector.memset(vh[:, :, :, :, D : D + 1], 1.0 / SA)
            nc.gpsimd.memset(vl[:, :, :, :, D : D + 1], 0.0)

            with (
                tc.tile_pool(name="scp2", bufs=3, space="PSUM") as scp2,
                tc.tile_pool(name="avp", bufs=2, space="PSUM") as avp,
                tc.tile_pool(name="exp", bufs=16) as expool,
                tc.tile_pool(name="nrm", bufs=4) as nrm,
                tc.tile_pool(name="yst", bufs=4) as yst,
            ):
                # prologue warmups: ACT Exp table preload + PE p-state ramp
                dm = nrm.tile([1, 2], F32, name="dm")
                nc.vector.memset(dm, 0.0)
                dm2 = nrm.tile([1, 2], F32, name="dm2")
                nc.scalar.activation(dm2, dm, EXP)
                nc.vector.memset(wup, 0.0)
                wps = scp2.tile([128, 512], F32, name="s2")
                for i in range(8):
                    nc.tensor.matmul(
                        wps, wup[:, 0:128], wup, start=(i == 0), stop=(i == 7)
                    )

                ex_t = {}
                av_t = {}

                def qk_proj(f):
                    # q-dims chunk f: one paired evac over both query halves
                    ps = scp2.tile([128, 1024], F32, name="s2")
                    for ih in range(2):
                        for cp in range(2):
                            nc.tensor.matmul(
                                ps[:, ih * 512 : (ih + 1) * 512],
                                w1[:, 0, cp, :, f * 128 : (f + 1) * 128],
                                xq[:, cp, :, ih * 512 : (ih + 1) * 512],
                                start=(cp == 0),
                                stop=(cp == 1),
                                perf_mode=DR,
                            )
                    nc.scalar.activation(
                        qt[:, f, 0, :],
                        ps,
                        IDENT,
                        bias=bqk[:, f : f + 1],
                        scale=1.0 / SW,
                    )
                def kt_proj(f, tp):
                    # k-dims chunk f, th pair tp: one paired evac
                    ps = scp2.tile([128, 1024], F32, name="s2")
                    for t2 in range(2):
                        th = tp * 2 + t2
                        for cp in range(2):
                            nc.tensor.matmul(
                                ps[:, t2 * 512 : (t2 + 1) * 512],
                                w1[:, 1, cp, :, f * 128 : (f + 1) * 128],
                                xk[:, 0, cp, :, th * 512 : (th + 1) * 512],
                                start=(cp == 0),
                                stop=(cp == 1),
                                perf_mode=DR,
                            )
                    # k bias is softmax-invariant (constant per query): skip
                    dst = kt[:, f, tp * 1024 : (tp + 1) * 1024]
                    if KT_PAT[(f * 2 + tp) % 2] == "D":
                        nc.vector.tensor_scalar(
                            dst, ps, 1.0 / SW, 0.0, AL.mult, AL.add
                        )
                    else:
                        nc.scalar.mul(dst, ps, 1.0 / SW)

                def v_proj(t):
                    # one seq chunk; PSUM from the av pool, which is idle
                    # during head 0, keeping scp2 free for the exp rotation
                    ps = avp.tile([128, 512], F32, name="av")
                    first = True
                    for xi, wi in ((0, 0), (0, 1), (1, 0)):
                        for cp in range(2):
                            nc.tensor.matmul(
                                ps,
                                xk[:, xi, cp, :, t * 128 : (t + 1) * 128],
                                w2[:, wi, cp],
                                start=first,
                                stop=(xi == 1 and cp == 1),
                                perf_mode=DR,
                            )
                            first = False
                    psv = ps.rearrange("p (h d) -> p h d", h=H)
                    p2, pl = t // 2, t % 2
                    hi = vh[:, p2, pl, :, 0:D]
                    nc.scalar.mul(hi, psv, 1.0 / SW)
                    nc.vector.scalar_tensor_tensor(
                        vl[:, p2, pl, :, 0:D], psv, 1.0 / SW, hi, AL.mult, AL.subtract
                    )

                def score(h, j):
                    f, base = h // 2, 64 * (h % 2)
                    jp, pl = j // 2, j % 2
                    if pl == 0:
                        ex_t[(h, jp)] = expool.tile([128, 2, LQ], F8, name="ex")
                    ex = ex_t[(h, jp)]
                    lhs = kt[base : base + 64, f, j * 128 : (j + 1) * 128]
                    lhs = lhs.unsqueeze(1).broadcast_to((64, 2, 128))
                    ps = scp2.tile([128, 1024], F32, name="s2")
                    for ih in range(2):
                        nc.tensor.matmul(
                            ps[:, ih * 512 : (ih + 1) * 512],
                            lhs,
                            qt[base : base + 64, f, :, ih * 512 : (ih + 1) * 512],
                            start=True,
                            stop=True,
                            perf_mode=DR,
                        )
                    if EXP_PAT[j] == "A":
                        nc.scalar.activation(ex[:, pl, :], ps, EXP, scale=1.0 / 8.0)
                    else:
                        nc.vector.tensor_scalar(
                            ex.bitcast(U8)[:, pl, :], ps, C1, C2, AL.mult, AL.add
                        )

                def attnv_qc(h, qc):
                    # one PSUM accumulation region at a time per bank: the
                    # 16 matmuls of a qc region run as one sequential group
                    # (hardware allows only one open accumulation group per
                    # PSUM bank; interleaving regions corrupts the sums).
                    if qc == 0:
                        av_t[h] = [
                            avp.tile([128, 4, D + 1], F32, name="av")
                            for _ in range(2)
                        ]
                    out = av_t[h][qc // 4][:, qc % 4, :]
                    for jp in range(8):
                        ex = ex_t[(h, jp)]
                        if debug and h == 0 and qc == 0:
                            nc.sync.dma_start(out=dex_d[jp], in_=ex)
                        exq = ex[:, :, qc * 128 : (qc + 1) * 128]
                        for gi, vv in enumerate((vh, vl)):
                            nc.tensor.matmul(
                                out,
                                exq,
                                vv[:, jp, :, h, :],
                                start=(jp == 0 and gi == 0),
                                stop=(jp == 7 and gi == 1),
                                perf_mode=DR,
                            )
                    if qc == 7:
                        for jp in range(8):
                            del ex_t[(h, jp)]

                nrm_t = {}

                def norm_half(h, half):
                    if half == 0:
                        nrm_t[h] = (
                            nrm.tile([128, 8], F32, name="rc"),
                            nrm.tile([128, 8, D], BF, name="avsb"),
                        )
                    rc, avsb = nrm_t[h]
                    av = av_t[h][half]
                    sl = slice(half * 4, half * 4 + 4)
                    nc.vector.reciprocal(rc[:, sl], av[:, :, D : D + 1])
                    nc.vector.tensor_copy(avsb[:, sl, :], av[:, :, 0:D])
                    for qc in range(half * 4, half * 4 + 4):
                        nc.gpsimd.tensor_scalar(
                            att[qc][:, h, :],
                            avsb[:, qc, :],
                            rc[:, qc : qc + 1],
                            1.0,
                            AL.mult,
                            AL.mult,
                        )
                    if half == 1:
                        del av_t[h]
                        if debug:
                            nc.sync.dma_start(out=davsb_d[h], in_=avsb)
                            nc.sync.dma_start(out=drc_d[h], in_=rc)
                        del nrm_t[h]

                def norm(h):
                    norm_half(h, 0)
                    norm_half(h, 1)

                def tp4(cc, g, pool=None):
                    # 4 transposes of head-pair cc, query chunks 4g..4g+3,
                    # evacuated in one [128, 512] op
                    tp = (pool or scp2).tile([128, 4, 128], BF, name="s2" if pool is None else "av")
                    for q2 in range(4):
                        qc = g * 4 + q2
                        nc.tensor.transpose(
                            tp[:, q2, :], att[qc][:, 2 * cc : 2 * cc + 2, :], idn
                        )
                    dst = atT[cc // 2][:, cc % 2, g * 512 : (g + 1) * 512]
                    if TEV_PAT[(cc * 2 + g) % 2] == "D":
                        nc.vector.tensor_copy(dst, tp)
                    else:
                        nc.scalar.copy(dst, tp)

                def out_proj(i):
                    # query chunk pair (2i, 2i+1): paired evac + 2 DMAs
                    ps = scp2.tile([128, 2, 512], F32, name="s2")
                    for pl in range(2):
                        t = i * 2 + pl
                        k = 0
                        for cp in range(2):
                            for p2 in range(2):
                                nc.tensor.matmul(
                                    ps[:, pl, :],
                                    atT[cp][:, p2, t * 128 : (t + 1) * 128],
                                    w3[:, 0, cp, p2, :],
                                    start=(k == 0),
                                    stop=False,
                                )
                                k += 1
                        nc.tensor.matmul(
                            ps[:, pl, :], ones_r, bo2, start=False, stop=True
                        )
                    ysb = yst.tile([128, 2, 512], F32, name="ysb")
                    if YEV_PAT[i % 2] == "D":
                        nc.vector.tensor_scalar(
                            ysb, ps, 1.0 / SA, 0.0, AL.mult, AL.add
                        )
                    else:
                        nc.scalar.mul(ysb, ps, 1.0 / SA)
                    for pl in range(2):
                        nc.sync.dma_start(out=y_d[i * 2 + pl], in_=ysb[:, pl, :])

                # ---- schedule --------------------------------------------
                qk_proj(0)
                kt_proj(0, 0)
                vq = list(range(16))

                def drain_v(n):
                    for _ in range(min(n, len(vq))):
                        v_proj(vq.pop(0))

                # attnv for head h runs during head h+1 (its exps done: no
                # PE stalls), one qc burst per j-slot.
                tpq = []  # pending tp4 units

                def norm_head(hh):
                    norm(hh)
                    if hh % 2 == 1:
                        tpq.append((hh // 2, 0))
                        tpq.append((hh // 2, 1))

                # (head, j) -> deferred projection work, spread over heads
                PROJ = {
                    (0, 5): lambda: kt_proj(0, 1),
                    (0, 13): lambda: qk_proj(1),
                    (1, 3): lambda: kt_proj(1, 0),
                    (1, 9): lambda: kt_proj(1, 1),
                    (2, 5): lambda: qk_proj(2),
                    (3, 3): lambda: kt_proj(2, 0),
                    (3, 9): lambda: kt_proj(2, 1),
                    (4, 5): lambda: qk_proj(3),
                    (5, 3): lambda: kt_proj(3, 0),
                    (5, 9): lambda: kt_proj(3, 1),
                }

                AQC = {2: 0, 3: 1, 4: 2, 5: 3, 8: 4, 9: 5, 10: 6, 11: 7}
                for h in range(8):
                    for j in range(16):
                        score(h, j)
                        if (h, j) in PROJ:
                            PROJ[(h, j)]()
                        if h == 0 and j in (5, 7, 9, 11, 13, 15):
                            drain_v(3)
                        if h >= 1 and j in AQC:
                            attnv_qc(h - 1, AQC[j])
                        if h >= 1 and j == 13:
                            norm_head(h - 1)
                        if h >= 1 and j in (14, 15) and tpq:
                            cc, g = tpq.pop(0)
                            tp4(cc, g, pool=avp)
                # tail: interleave the last head's attnv halves with the
                # first out-proj pair so the serial bf16 out-proj matmuls
                # overlap attnv(7, qc4-7) and the second norm half.
                assert not vq
                for qc in range(4):
                    attnv_qc(7, qc)
                norm_half(7, 0)
                for qc in range(4, 8):
                    attnv_qc(7, qc)
                norm_half(7, 1)
                for cc, g in tpq:
                    tp4(cc, g)
                tp4(3, 0)
                out_proj(0)
                tp4(3, 1)
                out_proj(1)
                out_proj(2)
                out_proj(3)
                if debug:
                    nc.sync.dma_start(out=dqt_d[:], in_=qt)
                    nc.sync.dma_start(out=dkt_d[:], in_=kt)
                    nc.sync.dma_start(out=dvh_d[:], in_=vh)
                    nc.sync.dma_start(out=dvl_d[:], in_=vl)
                    for qc in range(8):
                        nc.sync.dma_start(out=datt_d[qc], in_=att[qc])
                    for c in range(2):
                        nc.sync.dma_start(out=datT_d[c], in_=atT[c])

    nc.finalize()
    return nc


def _bo2(b):
    hi = b.astype(bfnp)
    lo = (b - hi.astype(np.float32)).astype(bfnp)
    return np.stack([hi, lo], axis=0).reshape(2, 512)


def _pairplane(a):
    # [512, N] -> [2, 128, 2, N] with c-chunk pairs interleaved on planes
    n = a.shape[1]
    return np.ascontiguousarray(a.reshape(2, 2, 128, n).transpose(0, 2, 1, 3))


def _prep_weights(qkv_w, qkv_b, out_w, out_b):
    w = qkv_w.reshape(H, 3, D, EMBED)
    b3 = qkv_b.reshape(H, 3, D)
    wq = w[:, 0].reshape(EMBED, EMBED)
    wk = w[:, 1].reshape(EMBED, EMBED)
    wv = w[:, 2].reshape(EMBED, EMBED)
    bq = b3[:, 0].reshape(EMBED)
    bv = b3[:, 2].reshape(EMBED)

    def hilo(m):
        hi = (m * SW).astype(f8np)
        lo = (m * SW - hi.astype(np.float32)).astype(f8np)
        return hi, lo

    wq8 = (wq.T.astype(np.float32) * SW).astype(f8np)
    wk8 = (wk.T.astype(np.float32) * SW).astype(f8np)
    wvh, wvl = hilo(wv.T.astype(np.float32))
    wo16 = out_w.T.astype(np.float32).astype(bfnp)

    def pack2(a, b):
        # two [512, 512] f8 -> [128, 2(which), 2(cp), 2(plane), 512]
        s = np.stack([_pairplane(a), _pairplane(b)], axis=0)  # [w, cp, 128, pl, f]
        return np.ascontiguousarray(s.transpose(2, 0, 1, 3, 4))

    bqk = np.zeros((128, 8), np.float32)
    for f in range(4):
        bqk[:, f] = bq[f * 128 : (f + 1) * 128]
    return {
        "w1": pack2(wq8, wk8),
        "w2": pack2(wvh, wvl),
        "w3": pack2(wo16, wo16),
        "bqk": bqk,
        "bo2": _bo2((out_b + out_w @ bv).astype(np.float32) * SA),
        "idn": np.eye(128, dtype=np.float32).astype(bfnp),
    }


def _make_in_maps(x1, x2, shared):
    xT = {}
    for mod, x in ((0, x1), (1, x2)):
        for b in range(B):
            t = np.ascontiguousarray(x[b].T).astype(np.float32)
            hi = t.astype(f8np)
            lo = (t - hi.astype(np.float32)).astype(f8np)
            xT[(mod, b)] = (hi, lo)
    in_maps = []
    for core in range(8):
        d, b, qh = core // 4, (core // 2) % 2, core % 2
        hi_q = xT[(d, b)][0][:, qh * LQ : (qh + 1) * LQ]
        hi_kv, lo_kv = xT[(1 - d, b)]
        m = dict(shared)
        m["xq"] = np.ascontiguousarray(
            _pairplane(np.ascontiguousarray(hi_q)).transpose(1, 0, 2, 3)
        )
        m["xk"] = np.ascontiguousarray(
            np.stack([_pairplane(hi_kv), _pairplane(lo_kv)], axis=0)
        )
        in_maps.append(m)
    return in_maps


def kernel(x1, x2, qkv_w, qkv_b, out_w, out_b):
    from concourse.bass_utils import run_bass_kernel_spmd

    x1 = np.asarray(x1, dtype=np.float32)
    x2 = np.asarray(x2, dtype=np.float32)
    shared = _prep_weights(
        np.asarray(qkv_w, np.float32),
        np.asarray(qkv_b, np.float32),
        np.asarray(out_w, np.float32),
        np.asarray(out_b, np.float32),
    )
    in_maps = _make_in_maps(x1, x2, shared)

    if "nc" not in _CACHE:
        _CACHE["nc"] = _build_nc()
    try:
        res = run_bass_kernel_spmd(_CACHE["nc"], in_maps, core_ids=list(range(8)))
    except Exception:
        res = run_bass_kernel_spmd(_CACHE["nc"], in_maps, core_ids=list(range(8)))

    out1 = np.empty((B, L, EMBED), np.float32)
    out2 = np.empty((B, L, EMBED), np.float32)
    outs = {0: out1, 1: out2}
    for core in range(8):
        d, b, qh = core // 4, (core // 2) % 2, core % 2
        yc = res.results[core]["y"].reshape(LQ, EMBED)
        outs[d][b, qh * LQ : (qh + 1) * LQ, :] = yc
    return out1, out2


# revision 47
# speedup vs baseline: 1.0726x; 1.0158x over previous
"""Cross-modal attention Trainium2 kernel (fp8 DoubleRow, pair-bank evac).

Sharding: 8 cores, one per (direction, batch, query-half):
  core = dir*4 + b*2 + qh
  dir 0: out1 rows (q from x1, k/v from x2); dir 1: out2 (q from x2, k/v from x1)

All GEMMs run in fp8e4m3 DoubleRow mode (2 contraction k-tiles per
instruction):
  - q/k/v projections contract over c in pairs of 128-chunks (host ships x
    and W with c-chunk pairs interleaved on the free axis).
  - scores per head contract over D=64 using a broadcast second plane.
  - attn@v streams per key-pair (jp) within its own head with exp'd scores
    stationary; v is split hi+lo fp8 (residual) for ~bf16 value accuracy.
    A separate tiny matmul against a 1/64 column accumulates the softmax
    denominator.
  - out-proj: att (bf16) stationary after PE transposes, wo bf16 moving;
    final evac folds 1/64 and the output bias.

The per-head pacing item is PSUM evacuation on ACT+DVE. All evacuations are
batched to [128, 1024] f32 reads spanning two PSUM banks (score pairs over
the two query halves; projection pairs over adjacent output chunks), which
amortizes the fixed SBUF/PSUM access latency per instruction.

exp is split A/D: ACT runs native Exp (scale=1/8) straight to fp8e4; DVE
runs a quantized-Schraudolph bit trick:
  u8 = round(score * log2(e)/8 * 8 + 55.54)  viewed as fp8e4 bits
Weights are prescaled x16 on host (e4m3 subnormal avoidance); 1/16 folds
into the PSUM evacuations. The 1/sqrt(D) score scale folds into the exp.
"""

import sys

sys.path.insert(0, "/opt/trn_rl_repo")

import numpy as np
import ml_dtypes

EMBED = 512
H = 8
D = 64
B = 2
L = 2048
LQ = 1024

SW = 16.0  # weight prescale (host)
SA = 64.0  # att prescale via 1/SA ones-column
C1 = float(np.log2(np.e) * 8.0 / 8.0)  # schraudolph mult (incl 1/sqrt(D))
C2 = 56.0 - 0.46  # schraudolph magic bias

f8np = ml_dtypes.float8_e4m3
bfnp = ml_dtypes.bfloat16

_CACHE = {}

# exp engine per (h, j): A=ACT native, D=DVE schraudolph. 9 A : 7 D.
EXP_PAT = "DAADADADADAADADA"
# schedule knobs (swept in sim)
TUNE = {
    "norm_j": 13,
    "aqc": (2, 3, 4, 5, 8, 9, 10, 11),
    "vd_slots": (5, 7, 9, 11, 13, 15),
    "vd_n": 3,
    "tp_slots": (14, 15),
}
# kt / tp4 / y evac engine rotation
KT_PAT = "DA"
TEV_PAT = "DA"
YEV_PAT = "DA"


def _build_nc(debug=False):
    import concourse.bacc as bacc
    import concourse.mybir as mybir
    import concourse.tile as tile

    BF = mybir.dt.bfloat16
    F32 = mybir.dt.float32
    F8 = mybir.dt.float8e4
    U8 = mybir.dt.uint8
    EXP = mybir.ActivationFunctionType.Exp
    IDENT = mybir.ActivationFunctionType.Identity
    DR = mybir.MatmulPerfMode.DoubleRow
    AL = mybir.AluOpType

    nc = bacc.Bacc("TRN2", target_bir_lowering=False)

    xq_d = nc.dram_tensor("xq", [128, 2, 2, LQ], F8, kind="ExternalInput")
    xk_d = nc.dram_tensor("xk", [2, 2, 128, 2, L], F8, kind="ExternalInput")
    w1_d = nc.dram_tensor("w1", [128, 2, 2, 2, 512], F8, kind="ExternalInput")
    w2_d = nc.dram_tensor("w2", [128, 2, 2, 2, 512], F8, kind="ExternalInput")
    w3_d = nc.dram_tensor("w3", [128, 2, 2, 2, 512], BF, kind="ExternalInput")
    bqk_d = nc.dram_tensor("bqk", [128, 8], F32, kind="ExternalInput")
    bo2_d = nc.dram_tensor("bo2", [2, 512], BF, kind="ExternalInput")
    idn_d = nc.dram_tensor("idn", [128, 128], BF, kind="ExternalInput")
    y_d = nc.dram_tensor("y", [8, 128, 512], F32, kind="ExternalOutput")
    if debug:
        dqt_d = nc.dram_tensor("dqt", [128, 4, 2, LQ], F8, kind="ExternalOutput")
        dkt_d = nc.dram_tensor("dkt", [128, 4, L], F8, kind="ExternalOutput")
        dvh_d = nc.dram_tensor("dvh", [128, 8, 2, H, D + 1], F8, kind="ExternalOutput")
        dvl_d = nc.dram_tensor("dvl", [128, 8, 2, H, D + 1], F8, kind="ExternalOutput")
        datt_d = nc.dram_tensor("datt", [8, 128, H, D], BF, kind="ExternalOutput")
        datT_d = nc.dram_tensor("datT", [2, 128, 2, LQ], BF, kind="ExternalOutput")
        dex_d = nc.dram_tensor("dex", [8, 128, 2, LQ], F8, kind="ExternalOutput")
        davsb_d = nc.dram_tensor("davsb", [8, 128, H, D], BF, kind="ExternalOutput")
        drc_d = nc.dram_tensor("drc", [8, 128, 8], F32, kind="ExternalOutput")

    with tile.TileContext(nc) as tc:
        with tc.tile_pool(name="persist", bufs=1) as pp:
            xq = pp.tile([128, 2, 2, LQ], F8, name="xq")
            xk = pp.tile([128, 2, 2, 2, L], F8, name="xk")
            w1 = pp.tile([128, 2, 2, 2, 512], F8, name="w1")
            w2 = pp.tile([128, 2, 2, 2, 512], F8, name="w2")
            w3 = pp.tile([128, 2, 2, 2, 512], BF, name="w3")
            bqk = pp.tile([128, 8], F32, name="bqk")
            bo2 = pp.tile([2, 512], BF, name="bo2")
            ones_r = pp.tile([2, 128], BF, name="ones_r")
            idn = pp.tile([128, 128], BF, name="idn")
            qt = pp.tile([128, 4, 2, LQ], F8, name="qt")
            kt = pp.tile([128, 4, L], F8, name="kt")
            # v carries a 65th column (1/SA on hi, 0 on lo) so the attn@v
            # matmul accumulates the softmax denominator in-place.
            vh = pp.tile([128, 8, 2, H, D + 1], F8, name="vh")
            vl = pp.tile([128, 8, 2, H, D + 1], F8, name="vl")
            att = [pp.tile([128, H, D], BF, name=f"att{qc}") for qc in range(8)]
            atT = [pp.tile([128, 2, LQ], BF, name=f"atT{c}") for c in range(2)]
            wup = pp.tile([128, 512], BF, name="wup")

            # DMA order tuned so qk_proj(0)'s inputs land first, then the
            # first v seq-half, then the second halves of everything. xk
            # quadrants are split in halves so the first k-proj and v-proj
            # can start before the full 16KB of x-other lands.
            # All input DMAs ride the sync (SP) queue: DMA dispatch holds the
            # issuing sequencer through descriptor generation, so issuing
            # from scalar/vector would block ACT/DVE compute dispatch.
            ha, hb = slice(0, 1024), slice(1024, 2048)
            nc.sync.dma_start(out=bqk, in_=bqk_d[:])
            nc.sync.dma_start(out=w1, in_=w1_d[:])
            nc.sync.dma_start(out=xq, in_=xq_d[:])
            hq, hr = slice(0, 512), slice(512, 1024)
            nc.sync.dma_start(out=xk[:, 0, 0, :, hq], in_=xk_d[0, 0, :, :, hq])
            nc.sync.dma_start(out=xk[:, 0, 1, :, hq], in_=xk_d[0, 1, :, :, hq])
            nc.sync.dma_start(out=xk[:, 0, 0, :, hr], in_=xk_d[0, 0, :, :, hr])
            nc.sync.dma_start(out=xk[:, 0, 1, :, hr], in_=xk_d[0, 1, :, :, hr])
            nc.sync.dma_start(out=xk[:, 0, 0, :, hb], in_=xk_d[0, 0, :, :, hb])
            nc.sync.dma_start(out=xk[:, 0, 1, :, hb], in_=xk_d[0, 1, :, :, hb])
            nc.sync.dma_start(out=w2, in_=w2_d[:])
            nc.sync.dma_start(out=xk[:, 1, 0, :, ha], in_=xk_d[1, 0, :, :, ha])
            nc.sync.dma_start(out=xk[:, 1, 1, :, ha], in_=xk_d[1, 1, :, :, ha])
            nc.sync.dma_start(out=xk[:, 1, 0, :, hb], in_=xk_d[1, 0, :, :, hb])
            nc.sync.dma_start(out=xk[:, 1, 1, :, hb], in_=xk_d[1, 1, :, :, hb])
            nc.gpsimd.memset(qt[:, :, 1, :], 0.0)
            nc.sync.dma_start(out=w3, in_=w3_d[:])
            nc.sync.dma_start(out=idn, in_=idn_d[:])
            nc.sync.dma_start(out=bo2, in_=bo2_d[:])
            nc.gpsimd.memset(ones_r, 1.0)
            nc.vector.memset(vh[:, :, :, :, D : D + 1], 1.0 / SA)
            nc.gpsimd.memset(vl[:, :, :, :, D : D + 1], 0.0)

            with (
                tc.tile_pool(name="scp2", bufs=3, space="PSUM") as scp2,
                tc.tile_pool(name="avp", bufs=2, space="PSUM") as avp,
                tc.tile_pool(name="exp", bufs=16) as expool,
                tc.tile_pool(name="nrm", bufs=4) as nrm,
                tc.tile_pool(name="yst", bufs=4) as yst,
            ):
                # prologue warmups: ACT Exp table preload + PE p-state ramp
                dm = nrm.tile([1, 2], F32, name="dm")
                nc.vector.memset(dm, 0.0)
                dm2 = nrm.tile([1, 2], F32, name="dm2")
                nc.scalar.activation(dm2, dm, EXP)

                ex_t = {}
                av_t = {}

                def qk_proj(f):
                    # q-dims chunk f: one paired evac over both query halves
                    ps = scp2.tile([128, 1024], F32, name="s2")
                    for ih in range(2):
                        for cp in range(2):
                            nc.tensor.matmul(
                                ps[:, ih * 512 : (ih + 1) * 512],
                                w1[:, 0, cp, :, f * 128 : (f + 1) * 128],
                                xq[:, cp, :, ih * 512 : (ih + 1) * 512],
                                start=(cp == 0),
                                stop=(cp == 1),
                                perf_mode=DR,
                            )
                    nc.scalar.activation(
                        qt[:, f, 0, :],
                        ps,
                        IDENT,
                        bias=bqk[:, f : f + 1],
                        scale=1.0 / SW,
                    )
                def kt_s(f, th):
                    # single-th k evac via the (idle) av pool: keeps scp2
                    # free for the exp rotation during the prologue
                    ps = avp.tile([128, 512], F32, name="av")
                    for cp in range(2):
                        nc.tensor.matmul(
                            ps,
                            w1[:, 1, cp, :, f * 128 : (f + 1) * 128],
                            xk[:, 0, cp, :, th * 512 : (th + 1) * 512],
                            start=(cp == 0),
                            stop=(cp == 1),
                            perf_mode=DR,
                        )
                    dst = kt[:, f, th * 512 : (th + 1) * 512]
                    if th % 2 == 0:
                        nc.vector.tensor_scalar(
                            dst, ps, 1.0 / SW, 0.0, AL.mult, AL.add
                        )
                    else:
                        nc.scalar.mul(dst, ps, 1.0 / SW)

                def kt_proj(f, tp):
                    # k-dims chunk f, th pair tp: one paired evac
                    ps = scp2.tile([128, 1024], F32, name="s2")
                    for t2 in range(2):
                        th = tp * 2 + t2
                        for cp in range(2):
                            nc.tensor.matmul(
                                ps[:, t2 * 512 : (t2 + 1) * 512],
                                w1[:, 1, cp, :, f * 128 : (f + 1) * 128],
                                xk[:, 0, cp, :, th * 512 : (th + 1) * 512],
                                start=(cp == 0),
                                stop=(cp == 1),
                                perf_mode=DR,
                            )
                    # k bias is softmax-invariant (constant per query): skip
                    dst = kt[:, f, tp * 1024 : (tp + 1) * 1024]
                    if KT_PAT[(f * 2 + tp) % 2] == "D":
                        nc.vector.tensor_scalar(
                            dst, ps, 1.0 / SW, 0.0, AL.mult, AL.add
                        )
                    else:
                        nc.scalar.mul(dst, ps, 1.0 / SW)

                def v_proj(t):
                    # one seq chunk; PSUM from the av pool, which is idle
                    # during head 0, keeping scp2 free for the exp rotation
                    ps = avp.tile([128, 512], F32, name="av")
                    first = True
                    for xi, wi in ((0, 0), (0, 1), (1, 0)):
                        for cp in range(2):
                            nc.tensor.matmul(
                                ps,
                                xk[:, xi, cp, :, t * 128 : (t + 1) * 128],
                                w2[:, wi, cp],
                                start=first,
                                stop=(xi == 1 and cp == 1),
                                perf_mode=DR,
                            )
                            first = False
                    psv = ps.rearrange("p (h d) -> p h d", h=H)
                    p2, pl = t // 2, t % 2
                    hi = vh[:, p2, pl, :, 0:D]
                    nc.scalar.mul(hi, psv, 1.0 / SW)
                    nc.vector.scalar_tensor_tensor(
                        vl[:, p2, pl, :, 0:D], psv, 1.0 / SW, hi, AL.mult, AL.subtract
                    )

                def score(h, j):
                    f, base = h // 2, 64 * (h % 2)
                    jp, pl = j // 2, j % 2
                    if pl == 0:
                        ex_t[(h, jp)] = expool.tile([128, 2, LQ], F8, name="ex")
                    ex = ex_t[(h, jp)]
                    lhs = kt[base : base + 64, f, j * 128 : (j + 1) * 128]
                    lhs = lhs.unsqueeze(1).broadcast_to((64, 2, 128))
                    ps = scp2.tile([128, 1024], F32, name="s2")
                    for ih in range(2):
                        nc.tensor.matmul(
                            ps[:, ih * 512 : (ih + 1) * 512],
                            lhs,
                            qt[base : base + 64, f, :, ih * 512 : (ih + 1) * 512],
                            start=True,
                            stop=True,
                            perf_mode=DR,
                        )
                    if EXP_PAT[j] == "A":
                        nc.scalar.activation(ex[:, pl, :], ps, EXP, scale=1.0 / 8.0)
                    else:
                        nc.vector.tensor_scalar(
                            ex.bitcast(U8)[:, pl, :], ps, C1, C2, AL.mult, AL.add
                        )

                def attnv_qc(h, qc):
                    # one PSUM accumulation region at a time per bank: the
                    # 16 matmuls of a qc region run as one sequential group
                    # (hardware allows only one open accumulation group per
                    # PSUM bank; interleaving regions corrupts the sums).
                    if qc == 0:
                        av_t[h] = [
                            avp.tile([128, 4, D + 1], F32, name="av")
                            for _ in range(2)
                        ]
                    out = av_t[h][qc // 4][:, qc % 4, :]
                    for jp in range(8):
                        ex = ex_t[(h, jp)]
                        if debug and h == 0 and qc == 0:
                            nc.sync.dma_start(out=dex_d[jp], in_=ex)
                        exq = ex[:, :, qc * 128 : (qc + 1) * 128]
                        for gi, vv in enumerate((vh, vl)):
                            nc.tensor.matmul(
                                out,
                                exq,
                                vv[:, jp, :, h, :],
                                start=(jp == 0 and gi == 0),
                                stop=(jp == 7 and gi == 1),
                                perf_mode=DR,
                            )
                    if qc == 7:
                        for jp in range(8):
                            del ex_t[(h, jp)]

                nrm_t = {}

                def norm_half(h, half):
                    if half == 0:
                        nrm_t[h] = (
                            nrm.tile([128, 8], F32, name="rc"),
                            nrm.tile([128, 8, D], BF, name="avsb"),
                        )
                    rc, avsb = nrm_t[h]
                    av = av_t[h][half]
                    sl = slice(half * 4, half * 4 + 4)
                    nc.vector.reciprocal(rc[:, sl], av[:, :, D : D + 1])
                    nc.vector.tensor_copy(avsb[:, sl, :], av[:, :, 0:D])
                    for qc in range(half * 4, half * 4 + 4):
                        nc.gpsimd.tensor_scalar(
                            att[qc][:, h, :],
                            avsb[:, qc, :],
                            rc[:, qc : qc + 1],
                            1.0,
                            AL.mult,
                            AL.mult,
                        )
                    if half == 1:
                        del av_t[h]
                        if debug:
                            nc.sync.dma_start(out=davsb_d[h], in_=avsb)
                            nc.sync.dma_start(out=drc_d[h], in_=rc)
                        del nrm_t[h]

                def norm(h):
                    norm_half(h, 0)
                    norm_half(h, 1)

                def tp4(cc, g, pool=None, eng=None):
                    # 4 transposes of head-pair cc, query chunks 4g..4g+3,
                    # evacuated in one [128, 512] op
                    tp = (pool or scp2).tile([128, 4, 128], BF, name="s2" if pool is None else "av")
                    for q2 in range(4):
                        qc = g * 4 + q2
                        nc.tensor.transpose(
                            tp[:, q2, :], att[qc][:, 2 * cc : 2 * cc + 2, :], idn
                        )
                    dst = atT[cc // 2][:, cc % 2, g * 512 : (g + 1) * 512]
                    if (eng or TEV_PAT[(cc * 2 + g) % 2]) == "D":
                        nc.vector.tensor_copy(dst, tp)
                    else:
                        nc.scalar.copy(dst, tp)

                def out_proj(i):
                    # query chunk pair (2i, 2i+1): paired evac + 2 DMAs
                    ps = scp2.tile([128, 2, 512], F32, name="s2")
                    for pl in range(2):
                        t = i * 2 + pl
                        k = 0
                        for cp in range(2):
                            for p2 in range(2):
                                nc.tensor.matmul(
                                    ps[:, pl, :],
                                    atT[cp][:, p2, t * 128 : (t + 1) * 128],
                                    w3[:, 0, cp, p2, :],
                                    start=(k == 0),
                                    stop=False,
                                )
                                k += 1
                        nc.tensor.matmul(
                            ps[:, pl, :], ones_r, bo2, start=False, stop=True
                        )
                    ysb = yst.tile([128, 2, 512], F32, name="ysb")
                    for pl in range(2):
                        if pl == 0:
                            nc.vector.tensor_scalar(
                                ysb[:, 0, :], ps[:, 0, :], 1.0 / SA, 0.0,
                                AL.mult, AL.add,
                            )
                        else:
                            nc.scalar.mul(ysb[:, 1, :], ps[:, 1, :], 1.0 / SA)
                        nc.sync.dma_start(out=y_d[i * 2 + pl], in_=ysb[:, pl, :])

                # ---- schedule --------------------------------------------
                qk_proj(0)
                kt_s(0, 0)
                kt_s(0, 1)
                vq = list(range(16))

                def drain_v(n):
                    for _ in range(min(n, len(vq))):
                        v_proj(vq.pop(0))

                # attnv for head h runs during head h+1 (its exps done: no
                # PE stalls), one qc burst per j-slot.
                tpq = []  # pending tp4 units

                def norm_head(hh):
                    norm(hh)
                    if hh % 2 == 1:
                        tpq.append((hh // 2, 0))
                        tpq.append((hh // 2, 1))

                # (head, j) -> deferred projection work, spread over heads
                PROJ = {
                    (0, 5): lambda: kt_proj(0, 1),
                    (0, 13): lambda: qk_proj(1),
                    (1, 3): lambda: kt_proj(1, 0),
                    (1, 9): lambda: kt_proj(1, 1),
                    (2, 5): lambda: qk_proj(2),
                    (3, 3): lambda: kt_proj(2, 0),
                    (3, 9): lambda: kt_proj(2, 1),
                    (4, 5): lambda: qk_proj(3),
                    (5, 3): lambda: kt_proj(3, 0),
                    (5, 9): lambda: kt_proj(3, 1),
                }

                AQC = {jj: i for i, jj in enumerate(TUNE["aqc"])}
                for h in range(8):
                    for j in range(16):
                        score(h, j)
                        if (h, j) in PROJ:
                            PROJ[(h, j)]()
                        if h == 0 and j in TUNE["vd_slots"]:
                            drain_v(TUNE["vd_n"])
                        if h >= 1 and j in AQC:
                            attnv_qc(h - 1, AQC[j])
                        if h >= 1 and j == TUNE["norm_j"]:
                            norm_head(h - 1)
                        if h >= 1 and j in TUNE["tp_slots"] and tpq:
                            cc, g = tpq.pop(0)
                            tp4(cc, g, pool=avp)
                # tail: interleave the last head's attnv halves with the
                # first out-proj pair so the serial bf16 out-proj matmuls
                # overlap attnv(7, qc4-7) and the second norm half.
                assert not vq
                for qc in range(4):
                    attnv_qc(7, qc)
                norm_half(7, 0)
                for qc in range(4, 8):
                    attnv_qc(7, qc)
                norm_half(7, 1)
                for cc, g in tpq:
                    tp4(cc, g)
                tp4(3, 0)
                out_proj(0)
                tp4(3, 1)
                out_proj(1)
                out_proj(2)
                out_proj(3)
                if debug:
                    nc.sync.dma_start(out=dqt_d[:], in_=qt)
                    nc.sync.dma_start(out=dkt_d[:], in_=kt)
                    nc.sync.dma_start(out=dvh_d[:], in_=vh)
                    nc.sync.dma_start(out=dvl_d[:], in_=vl)
                    for qc in range(8):
                        nc.sync.dma_start(out=datt_d[qc], in_=att[qc])
                    for c in range(2):
                        nc.sync.dma_start(out=datT_d[c], in_=atT[c])

    nc.finalize()
    return nc


def _bo2(b):
    hi = b.astype(bfnp)
    lo = (b - hi.astype(np.float32)).astype(bfnp)
    return np.stack([hi, lo], axis=0).reshape(2, 512)


def _pairplane(a):
    # [512, N] -> [2, 128, 2, N] with c-chunk pairs interleaved on planes
    n = a.shape[1]
    return np.ascontiguousarray(a.reshape(2, 2, 128, n).transpose(0, 2, 1, 3))


def _prep_weights(qkv_w, qkv_b, out_w, out_b):
    w = qkv_w.reshape(H, 3, D, EMBED)
    b3 = qkv_b.reshape(H, 3, D)
    wq = w[:, 0].reshape(EMBED, EMBED)
    wk = w[:, 1].reshape(EMBED, EMBED)
    wv = w[:, 2].reshape(EMBED, EMBED)
    bq = b3[:, 0].reshape(EMBED)
    bv = b3[:, 2].reshape(EMBED)

    def hilo(m):
        hi = (m * SW).astype(f8np)
        lo = (m * SW - hi.astype(np.float32)).astype(f8np)
        return hi, lo

    wq8 = (wq.T.astype(np.float32) * SW).astype(f8np)
    wk8 = (wk.T.astype(np.float32) * SW).astype(f8np)
    wvh, wvl = hilo(wv.T.astype(np.float32))
    wo16 = out_w.T.astype(np.float32).astype(bfnp)

    def pack2(a, b):
        # two [512, 512] f8 -> [128, 2(which), 2(cp), 2(plane), 512]
        s = np.stack([_pairplane(a), _pairplane(b)], axis=0)  # [w, cp, 128, pl, f]
        return np.ascontiguousarray(s.transpose(2, 0, 1, 3, 4))

    bqk = np.zeros((128, 8), np.float32)
    for f in range(4):
        bqk[:, f] = bq[f * 128 : (f + 1) * 128]
    return {
        "w1": pack2(wq8, wk8),
        "w2": pack2(wvh, wvl),
        "w3": pack2(wo16, wo16),
        "bqk": bqk,
        "bo2": _bo2((out_b + out_w @ bv).astype(np.float32) * SA),
        "idn": np.eye(128, dtype=np.float32).astype(bfnp),
    }


def _make_in_maps(x1, x2, shared):
    xT = {}
    for mod, x in ((0, x1), (1, x2)):
        for b in range(B):
            t = np.ascontiguousarray(x[b].T).astype(np.float32)
            hi = t.astype(f8np)
            lo = (t - hi.astype(np.float32)).astype(f8np)
            xT[(mod, b)] = (hi, lo)
    in_maps = []
    for core in range(8):
        d, b, qh = core // 4, (core // 2) % 2, core % 2
        hi_q = xT[(d, b)][0][:, qh * LQ : (qh + 1) * LQ]
        hi_kv, lo_kv = xT[(1 - d, b)]
        m = dict(shared)
        m["xq"] = np.ascontiguousarray(
            _pairplane(np.ascontiguousarray(hi_q)).transpose(1, 0, 2, 3)
        )
        m["xk"] = np.ascontiguousarray(
            np.stack([_pairplane(hi_kv), _pairplane(lo_kv)], axis=0)
        )
        in_maps.append(m)
    return in_maps


def kernel(x1, x2, qkv_w, qkv_b, out_w, out_b):
    from concourse.bass_utils import run_bass_kernel_spmd

    x1 = np.asarray(x1, dtype=np.float32)
    x2 = np.asarray(x2, dtype=np.float32)
    shared = _prep_weights(
        np.asarray(qkv_w, np.float32),
        np.asarray(qkv_b, np.float32),
        np.asarray(out_w, np.float32),
        np.asarray(out_b, np.float32),
    )
    in_maps = _make_in_maps(x1, x2, shared)

    if "nc" not in _CACHE:
        _CACHE["nc"] = _build_nc()
    try:
        res = run_bass_kernel_spmd(_CACHE["nc"], in_maps, core_ids=list(range(8)))
    except Exception:
        res = run_bass_kernel_spmd(_CACHE["nc"], in_maps, core_ids=list(range(8)))

    out1 = np.empty((B, L, EMBED), np.float32)
    out2 = np.empty((B, L, EMBED), np.float32)
    outs = {0: out1, 1: out2}
    for core in range(8):
        d, b, qh = core // 4, (core // 2) % 2, core % 2
        yc = res.results[core]["y"].reshape(LQ, EMBED)
        outs[d][b, qh * LQ : (qh + 1) * LQ, :] = yc
    return out1, out2
